# revision 23
# baseline (speedup 1.0000x reference)
"""Trainium2 Bass kernel for nn_BAAMamba (VMamba-style 4-direction Mamba classifier).

Sharding: pure data-parallel over batch — 8 cores x 1 image, each core runs the
full model on its image. No collectives.

Per-core pipeline (single NeuronCore):
  fp32 spine: patch-embed matmul (host-side im2col) -> pe LN -> block LNs,
  residual stream, CrossScan/CrossMerge permutation matmuls, head.
  bf16 mixer per (dir, depth):
    in_proj/x_proj/dt_proj/out_proj as bf16 PE matmuls in [d-part, t-free]
    causal depthwise conv on PE: 4 diag-matmul taps over zero-padded u,
      conv_b folded into the Silu bias (single PSUM consumer)
    u2/z gates via direct Silu on ACT (single act-table)
    softplus via Exp/Ln(bias=1) on ACT (shared natural_log_exp table)
    decay cube a[d,n,t]=exp(delta*A): 48 ACT exps with per-partition scale,
      chain-reset trick (a[:,n,0]=0) packs 16 n-chains into one flat scan
    selective scan: DVE tensor_tensor_scan, bf16 in/out (fp32 state)
    B_r/C_r broadcast cubes via bf16 DRAM round-trip (DMA, off the engines)
    g = h*C in place on DVE (bf16 2x mode)
    y = in-place binary-tree reduction over n (contiguous DVE adds),
      + u2*Dp folded in via scalar_tensor_tensor
    mixer emitted as two software-pipelined stages: stage1(block i+1)
      precedes stage2(block i) in every engine queue
"""

import os
import sys

import numpy as np

sys.path.insert(0, "/opt/trn_rl_repo")

import concourse.bass as bass  # noqa: E402
import concourse.bacc as bacc  # noqa: E402
import concourse.tile as tile  # noqa: E402
from concourse import mybir  # noqa: E402

F32 = mybir.dt.float32
BF16 = mybir.dt.bfloat16
AF = mybir.ActivationFunctionType
ALU = mybir.AluOpType

B = 8
IMG = 224
PATCH = 16
D = 192
DEPTH = 8
H = IMG // PATCH
W = H
L = H * W                      # 196
D_IN = 384
N_ST = 16                      # D_STATE
DT_R = 12
NCLS = 1000
EPS = 1e-5

TS = [(0, 128), (128, L - 128)]          # t tiles (offset, size)
KD = [(0, 128), (128, D - 128)]          # d=192 contraction tiles
NE = D_IN // 128                         # 3 e-tiles


def build_nc(ndirs=4, ndepth=DEPTH):
    nc = bacc.Bacc("TRN2")

    # ---- DRAM I/O ----
    xcol = nc.dram_tensor("xcol", (768, L), BF16, kind="ExternalInput")
    pwT = nc.dram_tensor("pwT", (768, D), BF16, kind="ExternalInput")
    pb = nc.dram_tensor("pb", (D,), F32, kind="ExternalInput")
    pe_g = nc.dram_tensor("pe_g", (D,), F32, kind="ExternalInput")
    pe_b = nc.dram_tensor("pe_b", (D,), F32, kind="ExternalInput")
    lnw = nc.dram_tensor("lnw", (4, DEPTH, D), F32, kind="ExternalInput")
    lnb = nc.dram_tensor("lnb", (4, DEPTH, D), F32, kind="ExternalInput")
    WinT = nc.dram_tensor("WinT", (4, DEPTH, D, 2 * D_IN), BF16, kind="ExternalInput")
    convd = nc.dram_tensor("convd", (4, DEPTH, NE, 4, 128, 128), BF16, kind="ExternalInput")
    convb = nc.dram_tensor("convb", (4, DEPTH, D_IN), F32, kind="ExternalInput")
    WxT = nc.dram_tensor("WxT", (4, DEPTH, D_IN, DT_R + 2 * N_ST), BF16, kind="ExternalInput")
    dtwT = nc.dram_tensor("dtwT", (4, DEPTH, DT_R, D_IN), BF16, kind="ExternalInput")
    dtb = nc.dram_tensor("dtb", (4, DEPTH, D_IN), F32, kind="ExternalInput")
    Aneg = nc.dram_tensor("Aneg", (4, DEPTH, D_IN, N_ST), F32, kind="ExternalInput")
    Dp = nc.dram_tensor("Dp", (4, DEPTH, D_IN), F32, kind="ExternalInput")
    WoT = nc.dram_tensor("WoT", (4, DEPTH, D_IN, D), BF16, kind="ExternalInput")
    onw = nc.dram_tensor("onw", (D,), F32, kind="ExternalInput")
    onb = nc.dram_tensor("onb", (D,), F32, kind="ExternalInput")
    hlw = nc.dram_tensor("hlw", (D,), F32, kind="ExternalInput")
    hlb = nc.dram_tensor("hlb", (D,), F32, kind="ExternalInput")
    hwT = nc.dram_tensor("hwT", (D, NCLS), F32, kind="ExternalInput")
    hb = nc.dram_tensor("hb", (NCLS,), F32, kind="ExternalInput")
    perm = nc.dram_tensor("perm", (4, L, L), F32, kind="ExternalInput")
    permI = nc.dram_tensor("permI", (4, L, L), F32, kind="ExternalInput")
    logits = nc.dram_tensor("logits", (1, NCLS), F32, kind="ExternalOutput")

    with tile.TileContext(nc) as tc:
        _emit(nc, tc, locals(), ndirs, ndepth)
    nc.compile()
    if not nc.is_finalized():
        nc.finalize()
    _merge_act_table_loads(nc)
    return nc


def _merge_act_table_loads(nc):
    """The auto-inserted act-table loads map Exp->exp_and_others(0) and
    Ln->natural_log(5), thrashing on every Exp<->Ln transition even though
    natural_log_exp_and_others(6) holds both. Retarget those loads to set 6
    and drop loads that become redundant (same set already resident)."""
    from concourse.hw_specs import get_activation_tables

    tables = list(get_activation_tables(nc.m.arch).items())
    AFT = mybir.ActivationFunctionType
    nl_exp = next(i for i, (name, s) in enumerate(tables)
                  if AFT.Exp in s and AFT.Ln in s)
    nl_set = tables[nl_exp][1]
    for blk in nc.main_func.blocks:
        # functions used between each load and the next load
        loads = []          # (idx in block, set of funcs used until next load)
        for idx, ins in enumerate(blk.instructions):
            if type(ins).__name__ == "InstLoadActFuncSet":
                loads.append((ins, set()))
            elif isinstance(ins, mybir.InstActivation) and loads:
                loads[-1][1].add(ins.func)
        for ins, funcs in loads:
            if funcs and funcs <= nl_set:
                ins.act_func_set_id = nl_exp
        cur = None
        keep = []
        for ins in blk.instructions:
            if type(ins).__name__ == "InstLoadActFuncSet":
                si = ins.sync_info
                no_sync = si is None or (not si.on_wait and not si.on_update)
                if cur == ins.act_func_set_id and no_sync:
                    continue
                cur = ins.act_func_set_id
            keep.append(ins)
        blk.instructions[:] = keep


def _emit(nc, tc, t_, ndirs, ndepth):
    from contextlib import ExitStack

    with ExitStack() as ctx:
        consts = ctx.enter_context(tc.tile_pool(name="consts", bufs=1))
        wpool = ctx.enter_context(tc.tile_pool(name="wpool", bufs=2))
        state = ctx.enter_context(tc.tile_pool(name="state", bufs=1))
        apool = ctx.enter_context(tc.tile_pool(name="apool", bufs=2))
        small = ctx.enter_context(tc.tile_pool(name="small", bufs=3))
        cpool = ctx.enter_context(tc.tile_pool(name="cpool", bufs=2))
        ps1 = ctx.enter_context(tc.tile_pool(name="ps1", bufs=6, space="PSUM"))
        dpool = ctx.enter_context(tc.tile_pool(name="dpool", bufs=2, space="DRAM"))

        # ---- constants ----
        from concourse.masks import make_identity

        ident = consts.tile([128, 128], F32)
        make_identity(nc, ident[:])

        P_sb = []
        PI_sb = []
        for di in range(4):
            p = consts.tile([128, 2, L], F32, tag=f"P{di}")
            pi = consts.tile([128, 2, L], F32, tag=f"PI{di}")
            for kt, (koff, ksz) in enumerate(TS):
                nc.sync.dma_start(p[:ksz, kt, :], t_["perm"][di, koff:koff + ksz, :])
                nc.sync.dma_start(pi[:ksz, kt, :], t_["permI"][di, koff:koff + ksz, :])
            P_sb.append(p)
            PI_sb.append(pi)

        # replicated [128, D] per-free-dim vectors
        def rep_vec(name):
            v = consts.tile([128, D], F32, tag=f"rep_{name}")
            nc.sync.dma_start(v[:], t_[name][:].unsqueeze(0).broadcast_to((128, D)))
            return v

        pb_r = rep_vec("pb")
        peg_r = rep_vec("pe_g")
        peb_r = rep_vec("pe_b")
        onw_r = rep_vec("onw")
        onb_r = rep_vec("onb")
        hlw_r = rep_vec("hlw")
        hlb_r = rep_vec("hlb")

        hb_sb = consts.tile([1, NCLS], F32)
        nc.sync.dma_start(hb_sb[:], t_["hb"][:].unsqueeze(0))
        hwT_sb = consts.tile([128, 2, NCLS], F32)
        for kd, (doff, dsz) in enumerate(KD):
            nc.sync.dma_start(hwT_sb[:dsz, kd, :], t_["hwT"][doff:doff + dsz, :])

        pwT_sb = consts.tile([128, 6, D], BF16)
        col_sb = consts.tile([128, 6, L], BF16)
        for kt in range(6):
            nc.sync.dma_start(pwT_sb[:, kt, :], t_["pwT"][kt * 128:(kt + 1) * 128, :])
            nc.sync.dma_start(col_sb[:, kt, :], t_["xcol"][kt * 128:(kt + 1) * 128, :])

        onescol = consts.tile([128, 1], F32)
        nc.vector.memset(onescol[:], 1.0 / L)
        eps_t = consts.tile([128, 1], F32)
        nc.vector.memset(eps_t[:], EPS)
        t_["eps_t"] = eps_t

        # ---- helpers ----
        def emit_ln(dst_xhat, src, tag):
            """src, dst: [128, 2, D] t-tiled activations; writes xhat (no affine)."""
            for tt, (toff, tsz) in enumerate(TS):
                st6 = small.tile([128, 6], F32, tag="bn6")
                mv = small.tile([128, 2], F32, tag="bn2")
                nc.vector.bn_stats(st6[:tsz], src[:tsz, tt, :])
                nc.vector.bn_aggr(mv[:tsz], st6[:tsz])
                lnv = small.tile([128, 1], F32, tag="lnv")
                rstd = small.tile([128, 1], F32, tag="rstd")
                nc.scalar.activation(lnv[:tsz], mv[:tsz, 1:2], AF.Ln, bias=eps_t[:tsz, :])
                nc.scalar.activation(rstd[:tsz], lnv[:tsz], AF.Exp, scale=-0.5)
                negmr = small.tile([128, 1], F32, tag="negmr")
                nc.vector.tensor_scalar(
                    out=negmr[:tsz], in0=mv[:tsz, 0:1],
                    scalar1=rstd[:tsz, 0:1], scalar2=-1.0,
                    op0=ALU.mult, op1=ALU.mult)
                nc.scalar.activation(dst_xhat[:tsz, tt, :], src[:tsz, tt, :],
                                     AF.Identity, scale=rstd[:tsz, 0:1],
                                     bias=negmr[:tsz, 0:1])

        # ---- patch embed ----
        feat_ln = state.tile([128, 2, D], F32, tag="feat_ln")
        for tt, (toff, tsz) in enumerate(TS):
            ps = ps1.tile([128, D], F32, tag="sps")
            for kt in range(6):
                nc.tensor.matmul(ps[:tsz, :], col_sb[:, kt, toff:toff + tsz],
                                 pwT_sb[:, kt, :], start=(kt == 0), stop=(kt == 5))
            # feat = psum + patch_b ; then pe-LN below
            nc.vector.tensor_add(feat_ln[:tsz, tt, :], ps[:tsz, :], pb_r[:tsz, :])
        xhat0 = state.tile([128, 2, D], F32, tag="xhat0")
        emit_ln(xhat0, feat_ln, "pe")
        # feat_ln = xhat * pe_g + pe_b
        for tt, (toff, tsz) in enumerate(TS):
            nc.vector.tensor_mul(feat_ln[:tsz, tt, :], xhat0[:tsz, tt, :], peg_r[:tsz, :])
            nc.vector.tensor_add(feat_ln[:tsz, tt, :], feat_ln[:tsz, tt, :], peb_r[:tsz, :])
        # shared depth-0 block-LN xhat of feat_ln
        emit_ln(xhat0, feat_ln, "blk0")

        # ---- per-direction state ----
        res_t = [state.tile([128, 2, D], F32, tag=f"res{di}", name=f"res{di}") for di in range(ndirs)]
        hid_t = [state.tile([128, 2, D], F32, tag=f"hid{di}", name=f"hid{di}") for di in range(ndirs)]

        for di in range(ndirs):
            for tt, (toff, tsz) in enumerate(TS):
                ps = ps1.tile([128, D], F32, tag="sps")
                for kt, (koff, ksz) in enumerate(TS):
                    nc.tensor.matmul(ps[:tsz, :], P_sb[di][:ksz, kt, toff:toff + tsz],
                                     feat_ln[:ksz, kt, :], start=(kt == 0), stop=(kt == 1))
                nc.scalar.copy(res_t[di][:tsz, tt, :], ps[:tsz, :])

        # ---- mixer blocks, software-pipelined in two stages ----
        # stage1(block i+1) is emitted before stage2(block i) so block i+1's
        # silu/softplus work sits ahead of block i's 48-exp a-cube batch in
        # the ACT queue (and its conv/proj work ahead in the DVE/PE queues).
        blocks = [(di, dep) for dep in range(ndepth) for di in range(ndirs)]
        pend = None          # (di, dep, st) awaiting stage2
        for di, dep in blocks:
            st = _emit_mixer(nc, tc, t_, di, dep, res_t[di], hid_t[di],
                             xhat0 if dep == 0 else None, P_sb[di], ident,
                             wpool, apool, small, cpool, ps1, dpool,
                             state)
            if pend is not None:
                pdi, pdep, pst = pend
                _emit_mixer_stage2(nc, tc, t_, pdi, pdep, hid_t[pdi], pst,
                                   wpool, apool, small, cpool, ps1, dpool,
                                   state)
            pend = (di, dep, st)
        pdi, pdep, pst = pend
        _emit_mixer_stage2(nc, tc, t_, pdi, pdep, hid_t[pdi], pst,
                           wpool, apool, small, cpool, ps1, dpool, state)

        # ---- final = hidden + residual ; CrossMerge ----
        for di in range(ndirs):
            for tt, (toff, tsz) in enumerate(TS):
                nc.vector.tensor_add(res_t[di][:tsz, tt, :], res_t[di][:tsz, tt, :],
                                     hid_t[di][:tsz, tt, :])
        merged = state.tile([128, 2, D], F32, tag="merged")
        for tt, (toff, tsz) in enumerate(TS):
            ps = ps1.tile([128, D], F32, tag="sps")
            nmm = ndirs * 2
            i = 0
            for di in range(ndirs):
                for kt, (koff, ksz) in enumerate(TS):
                    nc.tensor.matmul(ps[:tsz, :], PI_sb[di][:ksz, kt, toff:toff + tsz],
                                     res_t[di][:ksz, kt, :], start=(i == 0), stop=(i == nmm - 1))
                    i += 1
            nc.scalar.copy(merged[:tsz, tt, :], ps[:tsz, :])

        # ---- out_norm LN + head LN ----
        xh = state.tile([128, 2, D], F32, tag="xh_final")
        emit_ln(xh, merged, "on")
        for tt, (toff, tsz) in enumerate(TS):
            nc.vector.tensor_mul(merged[:tsz, tt, :], xh[:tsz, tt, :], onw_r[:tsz, :])
            nc.vector.tensor_add(merged[:tsz, tt, :], merged[:tsz, tt, :], onb_r[:tsz, :])
        emit_ln(xh, merged, "hl")
        for tt, (toff, tsz) in enumerate(TS):
            nc.vector.tensor_mul(merged[:tsz, tt, :], xh[:tsz, tt, :], hlw_r[:tsz, :])
            nc.vector.tensor_add(merged[:tsz, tt, :], merged[:tsz, tt, :], hlb_r[:tsz, :])

        # ---- mean pool (x 1/L via ones value) ----
        psp = ps1.tile([1, D], F32, tag="sps")
        for kt, (koff, ksz) in enumerate(TS):
            nc.tensor.matmul(psp[:, :], onescol[:ksz, :], merged[:ksz, kt, :],
                             start=(kt == 0), stop=(kt == 1))
        pooled = small.tile([1, D], F32, tag="pooled")
        nc.scalar.copy(pooled[:], psp[:])
        # transpose pooled [1, 192] -> [192, 1]
        pooledT = small.tile([128, 2, 1], F32, tag="pooledT")
        for kd, (doff, dsz) in enumerate(KD):
            pst = ps1.tile([128, 1], F32, tag="sps")
            nc.tensor.transpose(pst[:dsz, :], pooled[:, doff:doff + dsz], ident[:1, :1])
            nc.scalar.copy(pooledT[:dsz, kd, :], pst[:dsz, :])

        # ---- head ----
        log_sb = small.tile([1, NCLS], F32, tag="logsb")
        for half in range(2):
            psh = ps1.tile([1, 500], F32, tag="sps")
            for kd, (doff, dsz) in enumerate(KD):
                nc.tensor.matmul(psh[:, :], pooledT[:dsz, kd, :],
                                 hwT_sb[:dsz, kd, half * 500:(half + 1) * 500],
                                 start=(kd == 0), stop=(kd == 1))
            nc.vector.tensor_add(log_sb[:, half * 500:(half + 1) * 500], psh[:, :],
                                 hb_sb[:, half * 500:(half + 1) * 500])
        nc.sync.dma_start(t_["logits"][:], log_sb[:])


def _emit_mixer(nc, tc, t_, di, dep, res, hid, xhat0, P_di, ident,
                wpool, apool, small, cpool, ps1, dpool, state):
    # ---- stream weights (bf16 matmul weights; fp32 per-partition scalars) ----
    WinT_sb = wpool.tile([128, 2, 2 * D_IN], BF16, tag="WinT")
    for kd, (doff, dsz) in enumerate(KD):
        nc.sync.dma_start(WinT_sb[:dsz, kd, :], t_["WinT"][di, dep, doff:doff + dsz, :])
    WxT_sb = wpool.tile([128, NE, 44], BF16, tag="WxT")
    WoT_sb = wpool.tile([128, NE, D], BF16, tag="WoT")
    dtwT_sb = wpool.tile([DT_R, NE, 128], BF16, tag="dtwT")
    for ke in range(NE):
        nc.sync.dma_start(WxT_sb[:, ke, :], t_["WxT"][di, dep, ke * 128:(ke + 1) * 128, :])
        nc.sync.dma_start(WoT_sb[:, ke, :], t_["WoT"][di, dep, ke * 128:(ke + 1) * 128, :])
        nc.sync.dma_start(dtwT_sb[:, ke, :], t_["dtwT"][di, dep, :, ke * 128:(ke + 1) * 128])
    Aneg_sb = wpool.tile([128, NE, N_ST], F32, tag="Aneg")
    nc.sync.dma_start(Aneg_sb[:], t_["Aneg"][di, dep, :, :].rearrange("(a p) n -> p a n", p=128))
    convd_sb = wpool.tile([128, NE, 4, 128], BF16, tag="convd")
    nc.sync.dma_start(convd_sb[:], t_["convd"][di, dep].rearrange("e k p q -> p e k q"))
    convb_sb = wpool.tile([128, NE], F32, tag="convb")
    nc.sync.dma_start(convb_sb[:], t_["convb"][di, dep, :].rearrange("(a p) -> p a", p=128))
    dtb_sb = wpool.tile([128, NE], F32, tag="dtb")
    nc.sync.dma_start(dtb_sb[:], t_["dtb"][di, dep, :].rearrange("(a p) -> p a", p=128))
    Dp_sb = wpool.tile([128, NE], F32, tag="Dp")
    nc.sync.dma_start(Dp_sb[:], t_["Dp"][di, dep, :].rearrange("(a p) -> p a", p=128))
    lnw_sb = wpool.tile([128, 2], F32, tag="lnw")
    lnb_sb = wpool.tile([128, 2], F32, tag="lnb")
    for kd, (doff, dsz) in enumerate(KD):
        nc.sync.dma_start(lnw_sb[:dsz, kd:kd + 1], t_["lnw"][di, dep, doff:doff + dsz].unsqueeze(1))
        nc.sync.dma_start(lnb_sb[:dsz, kd:kd + 1], t_["lnb"][di, dep, doff:doff + dsz].unsqueeze(1))

    # ---- xlnT [d-part(2), L] bf16 ----
    xlnT = apool.tile([128, 2, L], BF16, tag="xlnT")
    if xhat0 is not None:
        # depth 0: permute shared xhat0 via P matmuls
        for kd, (doff, dsz) in enumerate(KD):
            ps = ps1.tile([128, L], F32, tag="sps")
            for kt, (koff, ksz) in enumerate(TS):
                nc.tensor.matmul(ps[:dsz, :], xhat0[:ksz, kt, doff:doff + dsz],
                                 P_di[:ksz, kt, :], start=(kt == 0), stop=(kt == 1))
            nc.scalar.activation(xlnT[:dsz, kd, :], ps[:dsz, :], AF.Identity,
                                 scale=lnw_sb[:dsz, kd:kd + 1],
                                 bias=lnb_sb[:dsz, kd:kd + 1])
    else:
        # residual += hidden ; LN ; transpose
        xhat = apool.tile([128, 2, D], F32, tag="xhat", bufs=1)
        nc.vector.tensor_add(res[:, :, :].rearrange("p a d -> p (a d)"),
                             res[:, :, :].rearrange("p a d -> p (a d)"),
                             hid[:, :, :].rearrange("p a d -> p (a d)"))
        for tt, (toff, tsz) in enumerate(TS):
            st6 = small.tile([128, 6], F32, tag="bn6")
            mv = small.tile([128, 2], F32, tag="bn2")
            nc.vector.bn_stats(st6[:tsz], res[:tsz, tt, :])
            nc.vector.bn_aggr(mv[:tsz], st6[:tsz])
            lnv = small.tile([128, 1], F32, tag="lnv")
            rstd = small.tile([128, 1], F32, tag="rstd")
            nc.scalar.activation(lnv[:tsz], mv[:tsz, 1:2], AF.Ln, bias=t_["eps_t"][:tsz, :])
            nc.scalar.activation(rstd[:tsz], lnv[:tsz], AF.Exp, scale=-0.5)
            negmr = small.tile([128, 1], F32, tag="negmr")
            nc.vector.tensor_scalar(
                out=negmr[:tsz], in0=mv[:tsz, 0:1],
                scalar1=rstd[:tsz, 0:1], scalar2=-1.0,
                op0=ALU.mult, op1=ALU.mult)
            nc.scalar.activation(xhat[:tsz, tt, :], res[:tsz, tt, :],
                                 AF.Identity, scale=rstd[:tsz, 0:1],
                                 bias=negmr[:tsz, 0:1])
        for kd, (doff, dsz) in enumerate(KD):
            ps = ps1.tile([128, L], F32, tag="sps")
            for tt, (toff, tsz) in enumerate(TS):
                nc.tensor.transpose(ps[:dsz, toff:toff + tsz],
                                    xhat[:tsz, tt, doff:doff + dsz], ident[:tsz, :tsz])
            nc.scalar.activation(xlnT[:dsz, kd, :], ps[:dsz, :], AF.Identity,
                                 scale=lnw_sb[:dsz, kd:kd + 1],
                                 bias=lnb_sb[:dsz, kd:kd + 1])

    # ---- in_proj + conv + silu(u) ; silu(z) directly from PSUM ----
    u2 = apool.tile([128, NE, L], BF16, tag="u2")
    sz = apool.tile([128, NE, L], BF16, tag="sz", bufs=2)
    upad = apool.tile([128, NE, 3 + L], BF16, tag="upad", bufs=1)
    nc.vector.memset(upad[:, :, 0:3], 0.0)
    for ec in range(6):
        ps = ps1.tile([128, L], F32, tag="sps")
        for kd, (doff, dsz) in enumerate(KD):
            nc.tensor.matmul(ps[:, :], WinT_sb[:dsz, kd, ec * 128:(ec + 1) * 128],
                             xlnT[:dsz, kd, :], start=(kd == 0), stop=(kd == 1))
        if ec < NE:
            # causal depthwise conv on PE: zero-padded u, 4 diag-matmul taps
            nc.scalar.copy(upad[:, ec, 3:3 + L], ps[:, :])
            psc = ps1.tile([128, L], F32, tag="sps")
            for j in range(4):
                nc.tensor.matmul(psc[:, :], convd_sb[:, ec, j, :],
                                 upad[:, ec, j:j + L], start=(j == 0), stop=(j == 3))
            # u2 = silu(conv + conv_b) straight from PSUM
            nc.scalar.activation(u2[:, ec, :], psc[:, :], AF.Silu,
                                 bias=convb_sb[:, ec:ec + 1])
        else:
            # sz = silu(z) straight from PSUM
            nc.scalar.activation(sz[:, ec - NE, :], ps[:, :], AF.Silu)

    # ---- x_proj (dt / B / C split to keep base partitions legal) ----
    splits = [(0, DT_R), (DT_R, N_ST), (DT_R + N_ST, N_ST)]
    dtm = apool.tile([DT_R, L], BF16, tag="dtm", bufs=1)
    Bm = apool.tile([N_ST, L], BF16, tag="Bm")
    Cm = apool.tile([N_ST, L], BF16, tag="Cm")
    for si, (soff, ssz) in enumerate(splits):
        psx = ps1.tile([N_ST, L], F32, tag="sps")
        for ke in range(NE):
            nc.tensor.matmul(psx[:ssz, :], WxT_sb[:, ke, soff:soff + ssz],
                             u2[:, ke, :], start=(ke == 0), stop=(ke == NE - 1))
        dst = (dtm, Bm, Cm)[si]
        nc.scalar.copy(dst[:ssz, :], psx[:ssz, :])

    # ---- delta (softplus, stays in natural_log_exp table) + v ----
    delta = apool.tile([128, NE, L], BF16, tag="delta", bufs=2)
    v = apool.tile([128, NE, L], BF16, tag="v", bufs=2)
    spe = apool.tile([128, NE, L], BF16, tag="spe", bufs=1)
    for ec in range(NE):
        psd = ps1.tile([128, L], F32, tag="sps")
        nc.tensor.matmul(psd[:, :], dtwT_sb[:, ec, :], dtm[:, :],
                         start=True, stop=True)
        nc.scalar.activation(spe[:, ec, :], psd[:, :], AF.Exp, bias=dtb_sb[:, ec:ec + 1])
        nc.scalar.activation(delta[:, ec, :], spe[:, ec, :], AF.Ln, bias=1.0)
    nc.vector.tensor_mul(v[:].rearrange("p e t -> p (e t)"),
                         delta[:].rearrange("p e t -> p (e t)"),
                         u2[:].rearrange("p e t -> p (e t)"))

    # ---- B_r / C_r broadcast via bf16 DRAM round-trip (DMA, off the engines) ----
    bc_dram = dpool.tile([2, N_ST * L], BF16, tag="bc_dram")
    nc.sync.dma_start(bc_dram[0:1, :].rearrange("a (n t) -> (a n) t", t=L), Bm[:, :])
    nc.sync.dma_start(bc_dram[1:2, :].rearrange("a (n t) -> (a n) t", t=L), Cm[:, :])
    B_r = cpool.tile([128, N_ST, L], BF16, tag="Brep", bufs=2)
    C_r = cpool.tile([128, N_ST, L], BF16, tag="Crep", bufs=2)
    nc.sync.dma_start(B_r[:].rearrange("p n t -> p (n t)"),
                      bc_dram[0:1, :].broadcast_to((128, N_ST * L)))
    nc.sync.dma_start(C_r[:].rearrange("p n t -> p (n t)"),
                      bc_dram[1:2, :].broadcast_to((128, N_ST * L)))

    # state handed to stage 2
    return dict(u2=u2, sz=sz, delta=delta, v=v, B_r=B_r, C_r=C_r,
                Aneg_sb=Aneg_sb, Dp_sb=Dp_sb, WoT_sb=WoT_sb)


def _emit_mixer_stage2(nc, tc, t_, di, dep, hid, st,
                       wpool, apool, small, cpool, ps1, dpool, state):
    u2, sz, delta, v = st["u2"], st["sz"], st["delta"], st["v"]
    B_r, C_r = st["B_r"], st["C_r"]
    Aneg_sb, Dp_sb, WoT_sb = st["Aneg_sb"], st["Dp_sb"], st["WoT_sb"]

    # ---- per e-tile: b cube, a cube, scan, g, tree-reduce y ----
    y_sb = apool.tile([128, NE, L], BF16, tag="ysb", bufs=2)
    for ec in range(NE):
        b_sb = cpool.tile([128, N_ST, L], BF16, tag="bcube", bufs=4, name=f"bcube{ec}")
        v_b = v[:, ec, :].unsqueeze(1).broadcast_to((128, N_ST, L))
        nc.vector.tensor_mul(b_sb[:], v_b, B_r[:])

        a_sb = cpool.tile([128, N_ST, L], BF16, tag="acube", bufs=4, name=f"acube{ec}")
        for n in range(N_ST):
            nc.scalar.activation(a_sb[:, n, :], delta[:, ec, :], AF.Exp,
                                 scale=Aneg_sb[:, ec, n:n + 1])
        nc.vector.memset(a_sb[:, :, 0:1], 0.0)
        h_sb = cpool.tile([128, N_ST, L], BF16, tag="hcube", bufs=4, name=f"hcube{ec}")
        nc.vector.tensor_tensor_scan(
            out=h_sb[:].rearrange("p n t -> p (n t)"),
            data0=a_sb[:].rearrange("p n t -> p (n t)"),
            data1=b_sb[:].rearrange("p n t -> p (n t)"),
            initial=0.0, op0=ALU.mult, op1=ALU.add)

        # g = h * C in place (DVE 2x), then in-place tree over n
        g_sb = h_sb
        nc.vector.tensor_mul(g_sb[:].rearrange("p n t -> p (n t)"),
                             h_sb[:].rearrange("p n t -> p (n t)"),
                             C_r[:].rearrange("p n t -> p (n t)"))
        nc.vector.tensor_add(g_sb[:, 0:8, :], g_sb[:, 0:8, :], g_sb[:, 8:16, :])
        nc.vector.tensor_add(g_sb[:, 0:4, :], g_sb[:, 0:4, :], g_sb[:, 4:8, :])
        nc.vector.tensor_add(g_sb[:, 0:2, :], g_sb[:, 0:2, :], g_sb[:, 2:4, :])
        nc.vector.tensor_add(g_sb[:, 0, :], g_sb[:, 0, :], g_sb[:, 1, :])
        # y2 = sum_n g + u2*Dp ; y3 = y2 * silu(z)
        nc.vector.scalar_tensor_tensor(
            out=y_sb[:, ec, :], in0=u2[:, ec, :], scalar=Dp_sb[:, ec:ec + 1],
            in1=g_sb[:, 0, :], op0=ALU.mult, op1=ALU.add)
    nc.vector.tensor_mul(y_sb[:].rearrange("p e t -> p (e t)"),
                         y_sb[:].rearrange("p e t -> p (e t)"),
                         sz[:].rearrange("p e t -> p (e t)"))

    # ---- out_proj -> hidden ----
    for tt, (toff, tsz) in enumerate(TS):
        pso = ps1.tile([128, D], F32, tag="sps")
        for ke in range(NE):
            nc.tensor.matmul(pso[:tsz, :], y_sb[:, ke, toff:toff + tsz], WoT_sb[:, ke, :],
                             start=(ke == 0), stop=(ke == NE - 1))
        nc.scalar.copy(hid[:tsz, tt, :], pso[:tsz, :])


# ============================== host side ==============================

_NC_CACHE = {}


def _get_nc():
    if "nc" not in _NC_CACHE:
        _NC_CACHE["nc"] = build_nc()
    return _NC_CACHE["nc"]


def _ml():
    import ml_dtypes
    return ml_dtypes


def _conv_diag(conv_w):
    """[4,8,384,4] -> [4,8,NE,4,128,128] with convd[...,j,d,d] = w[...,ec*128+d, j]
    (tap j reads upad column j+t = u[t+j-3], matching the (3,0)-padded conv)."""
    w = conv_w.reshape(4, DEPTH, NE, 128, 4)
    out = np.zeros((4, DEPTH, NE, 4, 128, 128), np.float32)
    d = np.arange(128)
    for j in range(4):
        out[:, :, :, j, d, d] = w[:, :, :, :, j]
    return out


def _perm_matrices():
    idx = np.arange(L).reshape(H, W)
    perm0 = idx.reshape(-1)
    perm1 = idx.T.reshape(-1)
    perms = [perm0, perm1, perm0[::-1].copy(), perm1[::-1].copy()]
    P = np.zeros((4, L, L), np.float32)
    PI = np.zeros((4, L, L), np.float32)
    for di, pm in enumerate(perms):
        P[di, pm, np.arange(L)] = 1.0       # seq[t'] = sum_t P[t,t'] feat[t]
        PI[di] = P[di].T                     # merged[t] = sum_t' PI[t',t] out[t']
    return P, PI


def prep_inputs(inputs):
    """Host-side layout prep. Returns (shared weight map, per-core xcol list)."""
    import ml_dtypes

    bf = ml_dtypes.bfloat16
    g = {k: np.ascontiguousarray(np.asarray(v, dtype=np.float32)) for k, v in inputs.items()}
    P, PI = _perm_matrices()
    shared = dict(
        pwT=np.ascontiguousarray(g["patch_w"].reshape(D, 768).T).astype(bf),
        pb=g["patch_b"], pe_g=g["pe_ln_w"], pe_b=g["pe_ln_b"],
        lnw=g["ln_w"], lnb=g["ln_b"],
        WinT=np.ascontiguousarray(g["in_proj_w"].transpose(0, 1, 3, 2)).astype(bf),
        convd=_conv_diag(g["conv_w"]).astype(bf), convb=g["conv_b"],
        WxT=np.ascontiguousarray(g["x_proj_w"].transpose(0, 1, 3, 2)).astype(bf),
        dtwT=np.ascontiguousarray(g["dt_w"].transpose(0, 1, 3, 2)).astype(bf),
        dtb=g["dt_b"],
        Aneg=np.ascontiguousarray(-np.exp(g["A_log"])),
        Dp=g["Dp"],
        WoT=np.ascontiguousarray(g["out_proj_w"].transpose(0, 1, 3, 2)).astype(bf),
        onw=g["out_norm_w"], onb=g["out_norm_b"],
        hlw=g["head_ln_w"], hlb=g["head_ln_b"],
        hwT=np.ascontiguousarray(g["head_w"].T), hb=g["head_b"],
        perm=P, permI=PI,
    )
    x = g["x"]
    xcols = []
    for b in range(x.shape[0]):
        xb = x[b]                                          # (3, 224, 224)
        c = xb.reshape(3, H, PATCH, W, PATCH)              # (3, i, pi, j, pj)
        col = c.transpose(0, 2, 4, 1, 3).reshape(768, L)   # (c,pi,pj),(i,j)
        xcols.append(np.ascontiguousarray(col).astype(_ml().bfloat16))
    return shared, xcols


def kernel(**inputs):
    from concourse.bass_utils import run_bass_kernel_spmd

    nc = _get_nc()
    shared, xcols = prep_inputs(inputs)
    nb = len(xcols)
    in_maps = [dict(shared, xcol=xcols[b]) for b in range(nb)]
    res = run_bass_kernel_spmd(nc, in_maps, core_ids=list(range(nb)))
    out = np.stack([res.results[b]["logits"][0] for b in range(nb)])
    return out.astype(np.float32)
